# revision 5
# baseline (speedup 1.0000x reference)
"""nn_AffineLog: batched 4x4 affine matrix-log projected onto the 7-dim CSO basis.

Closed-form algorithm (replaces the reference's 24-term Mercator series):
the input affines are exactly [[e^s R, t],[0,1]] with R a rotation, so
  L3x3 = s I + f (R - R^T),  f = asin(sin th)/(2 sin th)  (poly in sin^2)
  translation u' = psi(C) t with psi(x) = x/(e^x-1) Bernoulli series,
  reduced via Cayley-Hamilton (Omega^3 = -th^2 Omega) to
  u' = (A - D q) t + B (w x t) + D (w.t) w,   q = th^2.
Data-parallel over 8 NeuronCores; elementwise pipeline on DVE/ACT engines.
"""

import os

# Whole-tile dependency tracking: our tiles are read via ~20 strided channel
# views each; per-subtile tracking makes the slot-recycling DMA carry more
# sync-wait commands than the HW DGE descriptor supports.
os.environ.setdefault("BY_DEFAULT_DISABLE_SUBTILE_DEPS", "1")

import numpy as np

import concourse.bass as bass
import concourse.bacc as bacc
import concourse.mybir as mybir
from concourse.tile import TileContext
from concourse.bass_utils import run_bass_kernel_spmd

AF = mybir.ActivationFunctionType
OP = mybir.AluOpType
F32 = mybir.dt.float32

NCORES = 8
B = 2_000_000
P = 128
JPP = 1954                  # free-dim elements per partition per core
NC_ELEMS = P * JPP          # 250112 elements per core (total 2000896, pad 896)
TILES = (489, 489, 488, 488)

SQ2 = float(np.sqrt(2.0))
SQ3 = float(np.sqrt(3.0))
# f(z) = asin(x)/(2x), z = 4 x^2: f = FC0 + FC1 z + FC2 z^2 + FC3 z^3 + FC4 z^4
FC0 = 0.5
FC1 = 0.5 / 24.0
FC2 = 0.5 * (3.0 / 40.0) / 16.0
FC3 = 0.5 * (5.0 / 112.0) / 64.0
FC4 = 0.5 * (35.0 / 1152.0) / 256.0


def _build(jpp=JPP, tiles=TILES):
    # Bacc (not bare Bass): its finalize runs generate_event_semaphores,
    # which splits multi-sem waits — TRN2 instructions take at most 1 wait.
    nc = bacc.Bacc("TRN2", target_bir_lowering=False, debug=False)
    n_el = P * jpp
    x = nc.dram_tensor("x", (n_el, 16), F32, kind="ExternalInput")
    y = nc.dram_tensor("y", (n_el, 7), F32, kind="ExternalOutput")
    xv = x[:, :].rearrange("(p j) c -> p (j c)", p=P)
    yv = y[:, :].rearrange("(p j) c -> p (j c)", p=P)

    with TileContext(nc) as tc:
        with (
            tc.tile_pool(name="io", bufs=2) as iop,
            tc.tile_pool(name="tp", bufs=1) as tp,
        ):
            off = 0
            for nf in tiles:
                IN = iop.tile([P, nf * 16], F32, tag="in", name="tin")
                OUT = iop.tile([P, nf * 7], F32, tag="out", name="tout")
                nc.sync.dma_start(out=IN, in_=xv[:, off * 16:(off + nf) * 16])
                inv = IN.rearrange("p (j c) -> p j c", c=16)
                outv = OUT.rearrange("p (j c) -> p j c", c=7)

                def ch(c):
                    return inv[:, :, c]

                def och(c):
                    return outv[:, :, c]

                def T(nm):
                    return tp.tile([P, nf], F32, tag=nm, name=nm)

                def tt(o, a, b, op):
                    nc.vector.tensor_tensor(out=o, in0=a, in1=b, op=op)

                def ts(o, a, s1, s2):  # o = a*s1 + s2
                    nc.vector.tensor_scalar(
                        out=o, in0=a, scalar1=s1, scalar2=s2,
                        op0=OP.mult, op1=OP.add)

                def stt(o, a, s, b, op0, op1):  # o = (a op0 s) op1 b
                    nc.vector.scalar_tensor_tensor(
                        out=o, in0=a, scalar=s, in1=b, op0=op0, op1=op1)

                mul, add, sub = OP.mult, OP.add, OP.subtract
                t0, t1, t2 = ch(3), ch(7), ch(11)

                u = T("u"); v = T("v")
                # e^{2s} = |col0(M)|^2
                tt(u, ch(0), ch(0), mul)
                tt(v, ch(4), ch(4), mul)
                tt(u, u, v, add)
                tt(v, ch(8), ch(8), mul)
                tt(u, u, v, add)
                lnd2 = T("lnd2"); es = T("es"); es2 = T("es2"); s = T("s")
                nc.scalar.activation(out=lnd2, in_=u, func=AF.Ln)
                nc.scalar.activation(out=es, in_=lnd2, func=AF.Exp, scale=-0.5)
                nc.scalar.activation(out=es2, in_=lnd2, func=AF.Exp, scale=-1.0)
                nc.scalar.mul(s, lnd2, 0.5)
                nc.scalar.mul(och(6), lnd2, SQ3 / 2.0)

                a1 = T("a1"); a2 = T("a2"); a3 = T("a3")
                tt(a1, ch(1), ch(4), sub)
                tt(a2, ch(2), ch(8), sub)
                tt(a3, ch(6), ch(9), sub)
                # S = a1^2+a2^2+a3^2 ; z = e^{-2s} S = 4 sin^2 th
                tt(u, a1, a1, mul)
                tt(v, a2, a2, mul)
                tt(u, u, v, add)
                tt(v, a3, a3, mul)
                tt(u, u, v, add)
                z = T("z"); z2 = T("z2")
                tt(z, es2, u, mul)
                tt(z2, z, z, mul)
                # f = FC0 + FC1 z + z2*(FC2 + FC3 z + FC4 z2)
                p1 = T("p1"); p2 = T("p2"); f = T("f")
                ts(p1, z, FC1, FC0)
                ts(p2, z, FC3, FC2)
                stt(p2, z2, FC4, p2, mul, add)
                tt(v, z2, p2, mul)
                tt(f, p1, v, add)
                # qt = theta^2 = f^2 z ; g = f e^{-s}
                qt = T("qt"); g = T("g")
                tt(v, f, z, mul)
                tt(qt, f, v, mul)
                tt(g, f, es, mul)
                # rotation outputs
                stt(och(3), g, SQ2, a1, mul, mul)
                stt(och(4), g, SQ2, a2, mul, mul)
                stt(och(5), g, SQ2, a3, mul, mul)
                # psi coefficients A, B, D (polys in s, qt)
                s2 = T("s2"); s3 = T("s3")
                tt(s2, s, s, mul)
                tt(s3, s2, s, mul)
                tt(v, s2, s2, mul)      # s^4
                A = T("A"); Bc = T("Bc"); D = T("D"); Ap = T("Ap")
                ts(A, s, -0.5, 1.0)
                stt(A, s2, 1.0 / 12.0, A, mul, add)
                stt(A, v, -1.0 / 720.0, A, mul, add)
                ts(Bc, s, 1.0 / 6.0, -0.5)
                stt(Bc, s3, -1.0 / 180.0, Bc, mul, add)
                tt(v, s, qt, mul)
                stt(Bc, v, 1.0 / 180.0, Bc, mul, add)
                ts(D, s2, -1.0 / 120.0, 1.0 / 12.0)
                stt(D, qt, 1.0 / 720.0, D, mul, add)
                tt(v, D, qt, mul)
                tt(Ap, A, v, sub)
                # Bg = B g ; P3 = D g^2 dtil
                Bg = T("Bg"); P3 = T("P3")
                tt(Bg, Bc, g, mul)
                tt(v, g, g, mul)
                tt(P3, D, v, mul)
                r1 = T("r1"); r2 = T("r2")
                # dtil = -a3 t0 + a2 t1 - a1 t2
                tt(r1, a3, t0, mul)
                tt(r2, a2, t1, mul)
                tt(r1, r2, r1, sub)
                tt(r2, a1, t2, mul)
                tt(r1, r1, r2, sub)
                tt(P3, P3, r1, mul)
                q1 = T("q1"); q2 = T("q2"); w1 = T("w1"); w2 = T("w2")
                # u0 = Ap t0 + Bg (a2 t2 + a1 t1) - P3 a3
                tt(q1, a2, t2, mul)
                tt(q2, a1, t1, mul)
                tt(q1, q1, q2, add)
                tt(w1, Ap, t0, mul)
                tt(w2, Bg, q1, mul)
                tt(w1, w1, w2, add)
                stt(w2, P3, -1.0, a3, mul, mul)
                tt(och(0), w1, w2, add)
                # u1 = Ap t1 + Bg (a3 t2 - a1 t0) + P3 a2
                tt(q1, a3, t2, mul)
                tt(q2, a1, t0, mul)
                tt(q1, q1, q2, sub)
                tt(w1, Ap, t1, mul)
                tt(w2, Bg, q1, mul)
                tt(w1, w1, w2, add)
                tt(w2, P3, a2, mul)
                tt(och(1), w1, w2, add)
                # u2 = Ap t2 - Bg (a3 t1 + a2 t0) - P3 a1
                tt(q1, a3, t1, mul)
                tt(q2, a2, t0, mul)
                tt(q1, q1, q2, add)
                tt(w1, Ap, t2, mul)
                stt(w2, Bg, -1.0, q1, mul, mul)
                tt(w1, w1, w2, add)
                stt(w2, P3, -1.0, a1, mul, mul)
                tt(och(2), w1, w2, add)

                nc.sync.dma_start(out=yv[:, off * 7:(off + nf) * 7], in_=OUT)
                off += nf
    if not nc.is_finalized():
        nc.finalize()  # runs bacc passes: wait legalization, reg alloc, ...
    return nc


def _run(affine, trace=False):
    x = np.ascontiguousarray(affine.reshape(B, 16).astype(np.float32, copy=False))
    pad = NCORES * NC_ELEMS - B
    padblk = np.zeros((pad, 16), np.float32)
    padblk[:, [0, 5, 10, 15]] = 1.0  # identity affines -> log = 0
    data = np.concatenate([x, padblk], 0).reshape(NCORES, NC_ELEMS, 16)
    nc = _build()
    res = run_bass_kernel_spmd(
        nc,
        [{"x": data[i]} for i in range(NCORES)],
        core_ids=list(range(NCORES)),
        trace=trace,
    )
    y = np.concatenate([r["y"] for r in res.results], 0)[:B]
    return y, res


def kernel(affine):
    y, _ = _run(np.asarray(affine), trace=False)
    return y


# revision 6
# speedup vs baseline: 1.5074x; 1.5074x over previous
"""nn_AffineLog: batched 4x4 affine matrix-log projected onto the 7-dim CSO basis.

Closed-form algorithm (replaces the reference's 24-term Mercator series):
inputs are exactly [[e^s R, t],[0,1]] with R a rotation, so
  L3x3 = s I + f (R - R^T),  f = asin(x)/(2x), x = sin th  (poly in x^2)
  translation u' = psi(C) t, psi(x) = x/(e^x-1), reduced via
  Omega^3 = -th^2 Omega to u' = (A - D q) t + B (w x t) + D (w.t) w.
Data-parallel over 8 NeuronCores; elementwise pipeline on DVE/ACT engines
with custom fused DVE ops (quartic Horner, fused square-sums, psi coeffs).
"""

import os

os.environ.setdefault("BY_DEFAULT_DISABLE_SUBTILE_DEPS", "1")

import numpy as np

import concourse.bass as bass
import concourse.bacc as bacc
import concourse.mybir as mybir
from concourse.tile import TileContext
from concourse.bass_utils import run_bass_kernel_spmd
from concourse import dve_ops as dops
from concourse.dve_spec import (
    Spec, Src0, Src1, C0, C1, C2, C3, One, sq, _spill_c3_to_src1, lower,
    _has_src1,
)
from concourse.dve_uop import DveOpSpec

AF = mybir.ActivationFunctionType
OP = mybir.AluOpType
F32 = mybir.dt.float32

NCORES = 8
B = 2_000_000
P = 128
JPP = 1954                  # free-dim elements per partition per core
NC_ELEMS = P * JPP          # 250112 per core (total 2000896, pad 896)
TILES = (489, 489, 488, 488)

SQ2 = float(np.sqrt(2.0))
SQ3 = float(np.sqrt(3.0))
# f'(z) = 2*asin(x)/(2x) with z = 4x^2:  f' = 1 + c1 z + c2 z^2 + c3 z^3 + c4 z^4
FP_C1 = 1.0 / 24.0
FP_C2 = 2.0 * 0.5 * (3.0 / 40.0) / 16.0
FP_C3 = 2.0 * 0.5 * (5.0 / 112.0) / 64.0
FP_C4 = 2.0 * 0.5 * (35.0 / 1152.0) / 256.0


# --- custom fused DVE ops (registered into concourse.dve_ops at import) ----
def _register(name, body):
    if name in dops._SUB_OPCODE_FOR_NAME:
        return next(o for o in dops.OPS if o.name == name)
    dops._SUB_OPCODE_FOR_NAME[name] = dops._CUSTOM_DVE_ROW_BASE + len(dops.OPS)
    assert dops._SUB_OPCODE_FOR_NAME[name] < 0x20
    spec = Spec(body=body)
    lowered = DveOpSpec(
        name=name,
        opcode=dops._SUB_OPCODE_FOR_NAME[name],
        uops=lower(spec, ver="v3"),
        rd1_en=_has_src1(spec),
    )
    op = dops.DveOp(name=name, spec=spec, subdim=False,
                    uops_sha={"v3": lowered.sha("v3")})
    dops.OPS.append(op)
    dops.CUSTOM_DVE_SPECS[name] = spec
    return op


# sq(a) + sq(b)
OP_SQSUM = _register("ANT_AFL_SQSUM", sq(Src0) + sq(Src1))
# a + sq(b)
OP_ADDSQ = _register("ANT_AFL_ADDSQ", Src0 + sq(Src1))
# normalized quartic Horner: ((((z*C0 + C1)z + C2)z + C3)z + 1); C3 via in1 col
OP_POLY4 = _register(
    "ANT_AFL_POLY4",
    _spill_c3_to_src1(((((Src0 * C0 + C1) * Src0 + C2) * Src0 + C3) * Src0) + One),
)
# A(s) = ((C0*s*s + C1)*s + C2)*s + 1
_m2 = (Src0 * C0) * Src0
OP_ACOEF = _register("ANT_AFL_ACOEF", (((_m2 + C1) * Src0 + C2) * Src0) + One)
# B(s,q) = (s*C1 + C2) + (s^3 - s*q)*C0
_s2 = Src0 * Src0
OP_BCOEF = _register(
    "ANT_AFL_BCOEF", (Src0 * C1 + C2) + ((_s2 * Src0 - Src0 * Src1) * C0))
# D(s,q) = (s*s*C0 + q*C1) + C2
OP_DCOEF = _register(
    "ANT_AFL_DCOEF", ((Src0 * Src0) * C0 + Src1 * C1) + C2)
# qt = sq(f')*z*C0   (C0 = 1/4)
OP_QTH = _register("ANT_AFL_QTH", (sq(Src0) * Src1) * C0)
# Dg2 = D * sq(g)
OP_DG2 = _register("ANT_AFL_DG2", Src0 * sq(Src1))


def _build(jpp=JPP, tiles=TILES):
    nc = bacc.Bacc("TRN2", target_bir_lowering=False, debug=False)
    n_el = P * jpp
    x = nc.dram_tensor("x", (n_el, 16), F32, kind="ExternalInput")
    y = nc.dram_tensor("y", (n_el, 7), F32, kind="ExternalOutput")
    xv = x[:, :].rearrange("(p j) c -> p (j c)", p=P)
    yv = y[:, :].rearrange("(p j) c -> p (j c)", p=P)

    mul, add, sub = OP.mult, OP.add, OP.subtract

    with TileContext(nc) as tc:
        with (
            tc.tile_pool(name="cst", bufs=1) as cstp,
            tc.tile_pool(name="io", bufs=2) as iop,
            tc.tile_pool(name="tp", bufs=1) as tp,
        ):
            c1col = cstp.tile([P, 1], F32, name="c1col")
            nc.vector.memset(c1col, FP_C1)

            off = 0
            for nf in tiles:
                IN = iop.tile([P, nf * 16], F32, tag="in", name="tin")
                OUT = iop.tile([P, nf * 7], F32, tag="out", name="tout")
                nc.sync.dma_start(out=IN, in_=xv[:, off * 16:(off + nf) * 16])
                inv = IN.rearrange("p (j c) -> p j c", c=16)
                # t channels (3,7,11) as [p, 3, nf]
                tv = IN.rearrange("p (j f g) -> p g f j", g=4, f=4)[:, 3, 0:3, :]
                ov = OUT.rearrange("p (j c) -> p c j", c=7)

                def ch(c):
                    return inv[:, :, c]

                def T(nm, k=1):
                    return tp.tile([P, nf * k], F32, tag=nm, name=nm)

                def pl(t, i, k=1):  # plane slice i (k planes) of a planar tile
                    return t[:, i * nf:(i + k) * nf]

                def pl3(t):  # planar [p, 3nf] -> [p, 3, nf]
                    return t.rearrange("p (c j) -> p c j", c=3)

                def bc3(a):  # [p, nf] -> broadcast [p, 3, nf]
                    return a.rearrange("p (o j) -> p o j", o=1).to_broadcast(
                        [P, 3, nf])

                def tt(o, a, b, op):
                    nc.vector.tensor_tensor(out=o, in0=a, in1=b, op=op)

                def stt(o, a, s, b, op0, op1):
                    nc.vector.scalar_tensor_tensor(
                        out=o, in0=a, scalar=s, in1=b, op0=op0, op1=op1)

                def cust(op_, o, a, b=None, s0=0.0, s1=0.0, imm2=0.0):
                    nc.vector._custom_dve(
                        op_, out=o, in0=a, in1=b, s0=s0, s1=s1, imm2=imm2)

                u = T("u"); v = T("v")
                # e^{2s} = |col0(M)|^2
                cust(OP_SQSUM, u, ch(0), ch(4))
                e2s = T("e2s")
                cust(OP_ADDSQ, e2s, u, ch(8))
                lnd2 = T("lnd2"); es = T("es"); es2 = T("es2"); s = T("s")
                nc.scalar.activation(out=lnd2, in_=e2s, func=AF.Ln)
                nc.scalar.activation(out=es, in_=lnd2, func=AF.Exp, scale=-0.5)
                nc.scalar.activation(out=es2, in_=lnd2, func=AF.Exp, scale=-1.0)
                nc.scalar.mul(s, lnd2, 0.5)
                nc.scalar.mul(ov[:, 6, :], lnd2, SQ3 / 2.0)

                A3 = T("A3", 3)
                tt(pl(A3, 0), ch(1), ch(4), sub)   # a1 = m01 - m10
                tt(pl(A3, 1), ch(2), ch(8), sub)   # a2 = m02 - m20
                tt(pl(A3, 2), ch(6), ch(9), sub)   # a3 = m12 - m21
                cust(OP_SQSUM, v, pl(A3, 0), pl(A3, 1))
                S = T("S")
                cust(OP_ADDSQ, S, v, pl(A3, 2))
                z = T("z")
                tt(z, es2, S, mul)                  # z = 4 sin^2 th
                fp = T("fp")
                cust(OP_POLY4, fp, z, c1col, s0=FP_C4, s1=FP_C3, imm2=FP_C2)
                qt = T("qt")
                cust(OP_QTH, qt, fp, z, s0=0.25)    # th^2
                g = T("g")
                stt(g, fp, 0.5, es, mul, mul)       # g = f e^{-s}
                # rotation outputs 3..5 = sqrt2 * g * a_k
                stt(ov[:, 3:6, :], bc3(g), SQ2, pl3(A3), mul, mul)
                # psi coefficients
                A = T("A"); Bc = T("Bc"); D = T("D")
                cust(OP_ACOEF, A, s,
                     s0=-1.0 / 720.0, s1=1.0 / 12.0, imm2=-0.5)
                cust(OP_BCOEF, Bc, s, qt,
                     s0=-1.0 / 180.0, s1=1.0 / 6.0, imm2=-0.5)
                cust(OP_DCOEF, D, s, qt,
                     s0=-1.0 / 120.0, s1=1.0 / 720.0, imm2=1.0 / 12.0)
                v2 = T("v2"); Ap = T("Ap"); Bg = T("Bg"); Dg2 = T("Dg2")
                tt(v2, D, qt, mul)
                tt(Ap, A, v2, sub)
                tt(Bg, Bc, g, mul)
                cust(OP_DG2, Dg2, D, g)
                # all 9 products P[i,j] = a_i * t_j at plane 3i+j
                P9 = T("P9", 9)
                for i in range(3):
                    tt(pl3(pl(P9, 3 * i, 3)), bc3(pl(A3, i)), tv, mul)
                # cross (ctil) and dot (dtil) from P9 planes
                C3t = T("C3t", 3)
                tt(pl(C3t, 0), pl(P9, 1), pl(P9, 5), add)       # a1t1+a2t2
                tt(pl(C3t, 1), pl(P9, 8), pl(P9, 0), sub)       # a3t2-a1t0
                stt(pl(C3t, 2), pl(P9, 7), -1.0, pl(P9, 3), mul, sub)
                dA = T("dA"); dt = T("dt"); P3 = T("P3")
                tt(dA, pl(P9, 4), pl(P9, 6), sub)               # a2t1-a3t0
                tt(dt, dA, pl(P9, 2), sub)                      # - a1t2
                tt(P3, Dg2, dt, mul)
                # w1 = Ap*t ; w2 = Bg*ctil ; pw = P3*(-a3,+a2,-a1)
                W1 = T("W1", 3); W2 = T("W2", 3); PW = T("PW", 3)
                tt(pl3(W1), bc3(Ap), tv, mul)
                tt(pl3(W2), bc3(Bg), pl3(C3t), mul)
                stt(pl(PW, 0), P3, -1.0, pl(A3, 2), mul, mul)
                tt(pl(PW, 1), P3, pl(A3, 1), mul)
                stt(pl(PW, 2), P3, -1.0, pl(A3, 0), mul, mul)
                tt(pl3(W1), pl3(W1), pl3(W2), add)
                tt(ov[:, 0:3, :], pl3(W1), pl3(PW), add)

                nc.sync.dma_start(out=yv[:, off * 7:(off + nf) * 7], in_=OUT)
                off += nf
    if not nc.is_finalized():
        nc.finalize()
    return nc


def _run(affine, trace=False):
    x = np.ascontiguousarray(affine.reshape(B, 16).astype(np.float32, copy=False))
    pad = NCORES * NC_ELEMS - B
    padblk = np.zeros((pad, 16), np.float32)
    padblk[:, [0, 5, 10, 15]] = 1.0  # identity affines -> log = 0
    data = np.concatenate([x, padblk], 0).reshape(NCORES, NC_ELEMS, 16)
    nc = _build()
    res = run_bass_kernel_spmd(
        nc,
        [{"x": data[i]} for i in range(NCORES)],
        core_ids=list(range(NCORES)),
        trace=trace,
    )
    y = np.concatenate([r["y"] for r in res.results], 0)[:B]
    return y, res


def kernel(affine):
    y, _ = _run(np.asarray(affine), trace=False)
    return y


# revision 9
# speedup vs baseline: 1.5287x; 1.0141x over previous
"""nn_AffineLog: batched 4x4 affine matrix-log projected onto the 7-dim CSO basis.

Closed-form algorithm (replaces the reference's 24-term Mercator series):
inputs are exactly [[e^s R, t],[0,1]] with R a rotation, so
  L3x3 = s I + f (R - R^T),  f = asin(x)/(2x), x = sin th  (poly in x^2)
  translation u' = psi(C) t, psi(x) = x/(e^x-1), reduced via
  Omega^3 = -th^2 Omega to u' = (A - D q) t + B (w x t) + D (w.t) w.
Data-parallel over 8 NeuronCores; elementwise pipeline on DVE/ACT engines
with custom fused DVE ops (quartic Horner, fused square-sums, psi coeffs).
"""

import os

os.environ.setdefault("BY_DEFAULT_DISABLE_SUBTILE_DEPS", "1")

import numpy as np

import concourse.bass as bass
import concourse.bacc as bacc
import concourse.mybir as mybir
from concourse.tile import TileContext
from concourse.bass_utils import run_bass_kernel_spmd
from concourse import dve_ops as dops
from concourse.dve_spec import (
    Spec, Src0, Src1, C0, C1, C2, C3, One, sq, _spill_c3_to_src1, lower,
    _has_src1,
)
from concourse.dve_uop import DveOpSpec

AF = mybir.ActivationFunctionType
OP = mybir.AluOpType
F32 = mybir.dt.float32

NCORES = 8
B = 2_000_000
P = 128
JPP = 1954                  # free-dim elements per partition per core
NC_ELEMS = P * JPP          # 250112 per core (total 2000896, pad 896)
TILES = (160, 620, 620, 554)

SQ2 = float(np.sqrt(2.0))
SQ3 = float(np.sqrt(3.0))
# f'(z) = 2*asin(x)/(2x) with z = 4x^2:  f' = 1 + c1 z + c2 z^2 + c3 z^3 + c4 z^4
FP_C1 = 1.0 / 24.0
FP_C2 = 2.0 * 0.5 * (3.0 / 40.0) / 16.0
FP_C3 = 2.0 * 0.5 * (5.0 / 112.0) / 64.0
FP_C4 = 2.0 * 0.5 * (35.0 / 1152.0) / 256.0


# --- custom fused DVE ops (registered into concourse.dve_ops at import) ----
def _register(name, body):
    if name in dops._SUB_OPCODE_FOR_NAME:
        return next(o for o in dops.OPS if o.name == name)
    dops._SUB_OPCODE_FOR_NAME[name] = dops._CUSTOM_DVE_ROW_BASE + len(dops.OPS)
    assert dops._SUB_OPCODE_FOR_NAME[name] < 0x20
    spec = Spec(body=body)
    lowered = DveOpSpec(
        name=name,
        opcode=dops._SUB_OPCODE_FOR_NAME[name],
        uops=lower(spec, ver="v3"),
        rd1_en=_has_src1(spec),
    )
    op = dops.DveOp(name=name, spec=spec, subdim=False,
                    uops_sha={"v3": lowered.sha("v3")})
    dops.OPS.append(op)
    dops.CUSTOM_DVE_SPECS[name] = spec
    return op


# sq(a) + sq(b)
OP_SQSUM = _register("ANT_AFL_SQSUM", sq(Src0) + sq(Src1))
# a + sq(b)
OP_ADDSQ = _register("ANT_AFL_ADDSQ", Src0 + sq(Src1))
# normalized quartic Horner: ((((z*C0 + C1)z + C2)z + C3)z + 1); C3 via in1 col
OP_POLY4 = _register(
    "ANT_AFL_POLY4",
    _spill_c3_to_src1(((((Src0 * C0 + C1) * Src0 + C2) * Src0 + C3) * Src0) + One),
)
# A(s) = ((C0*s*s + C1)*s + C2)*s + 1
_m2 = (Src0 * C0) * Src0
OP_ACOEF = _register("ANT_AFL_ACOEF", (((_m2 + C1) * Src0 + C2) * Src0) + One)
# B(s,q) = (s*C1 + C2) + (s^3 - s*q)*C0
_s2 = Src0 * Src0
OP_BCOEF = _register(
    "ANT_AFL_BCOEF", (Src0 * C1 + C2) + ((_s2 * Src0 - Src0 * Src1) * C0))
# D(s,q) = (s*s*C0 + q*C1) + C2
OP_DCOEF = _register(
    "ANT_AFL_DCOEF", ((Src0 * Src0) * C0 + Src1 * C1) + C2)
# qt = sq(f')*z*C0   (C0 = 1/4)
OP_QTH = _register("ANT_AFL_QTH", (sq(Src0) * Src1) * C0)
# Dg2 = D * sq(g)
OP_DG2 = _register("ANT_AFL_DG2", Src0 * sq(Src1))


def _build(jpp=JPP, tiles=TILES):
    nc = bacc.Bacc("TRN2", target_bir_lowering=False, debug=False)
    n_el = P * jpp
    x = nc.dram_tensor("x", (n_el, 16), F32, kind="ExternalInput")
    y = nc.dram_tensor("y", (n_el, 7), F32, kind="ExternalOutput")
    xv = x[:, :].rearrange("(p j) c -> p (j c)", p=P)
    yv = y[:, :].rearrange("(p j) c -> p (j c)", p=P)

    mul, add, sub = OP.mult, OP.add, OP.subtract

    with TileContext(nc) as tc:
        with (
            tc.tile_pool(name="cst", bufs=1) as cstp,
            tc.tile_pool(name="io", bufs=2) as iop,
            tc.tile_pool(name="tp", bufs=1) as tp,
        ):
            c1col = cstp.tile([P, 1], F32, name="c1col")
            nc.vector.memset(c1col, FP_C1)

            off = 0
            for nf in tiles:
                IN = iop.tile([P, nf * 16], F32, tag="in", name="tin")
                OUT = iop.tile([P, nf * 7], F32, tag="out", name="tout")
                nc.sync.dma_start(out=IN, in_=xv[:, off * 16:(off + nf) * 16])
                inv = IN.rearrange("p (j c) -> p j c", c=16)
                # t channels (3,7,11) as [p, 3, nf]
                tv = IN.rearrange("p (j f g) -> p g f j", g=4, f=4)[:, 3, 0:3, :]
                ov = OUT.rearrange("p (j c) -> p c j", c=7)

                def ch(c):
                    return inv[:, :, c]

                def T(nm, k=1):
                    return tp.tile([P, nf * k], F32, tag=nm, name=nm)

                def pl(t, i, k=1):  # plane slice i (k planes) of a planar tile
                    return t[:, i * nf:(i + k) * nf]

                def pl3(t):  # planar [p, 3nf] -> [p, 3, nf]
                    return t.rearrange("p (c j) -> p c j", c=3)

                def bc3(a):  # [p, nf] -> broadcast [p, 3, nf]
                    return a.rearrange("p (o j) -> p o j", o=1).to_broadcast(
                        [P, 3, nf])

                def tt(o, a, b, op):
                    nc.vector.tensor_tensor(out=o, in0=a, in1=b, op=op)

                def stt(o, a, s, b, op0, op1):
                    nc.vector.scalar_tensor_tensor(
                        out=o, in0=a, scalar=s, in1=b, op0=op0, op1=op1)

                def cust(op_, o, a, b=None, s0=0.0, s1=0.0, imm2=0.0):
                    nc.vector._custom_dve(
                        op_, out=o, in0=a, in1=b, s0=s0, s1=s1, imm2=imm2)

                u = T("u"); v = T("v")
                # e^{2s} = |col0(M)|^2 ; ACT chain starts right after
                cust(OP_SQSUM, u, ch(0), ch(4))
                e2s = T("e2s")
                cust(OP_ADDSQ, e2s, u, ch(8))
                lnd2 = T("lnd2"); es = T("es"); es2 = T("es2"); s = T("s")
                nc.scalar.activation(out=lnd2, in_=e2s, func=AF.Ln)
                nc.scalar.activation(out=es, in_=lnd2, func=AF.Exp, scale=-0.5)
                nc.scalar.activation(out=es2, in_=lnd2, func=AF.Exp, scale=-1.0)
                nc.scalar.mul(s, lnd2, 0.5)
                nc.scalar.mul(ov[:, 6, :], lnd2, SQ3 / 2.0)

                A3 = T("A3", 3)
                tt(pl(A3, 0), ch(1), ch(4), sub)   # a1 = m01 - m10
                tt(pl(A3, 1), ch(2), ch(8), sub)   # a2 = m02 - m20
                tt(pl(A3, 2), ch(6), ch(9), sub)   # a3 = m12 - m21
                cust(OP_SQSUM, v, pl(A3, 0), pl(A3, 1))
                S = T("S")
                cust(OP_ADDSQ, S, v, pl(A3, 2))
                # all 9 products P[i,j] = a_i * t_j at plane 3i+j
                # (independent of the ACT chain -> hides its latency)
                P9 = T("P9", 9)
                for i in range(3):
                    tt(pl3(pl(P9, 3 * i, 3)), bc3(pl(A3, i)), tv, mul)
                # dtil, then cross (ctil) into the P9 planes dtil consumed
                dA = T("dA"); dt = T("dt")
                tt(dA, pl(P9, 4), pl(P9, 6), sub)               # a2t1-a3t0
                tt(dt, dA, pl(P9, 2), sub)                      # - a1t2
                cx = pl(P9, 2); cy = pl(P9, 4); cz = pl(P9, 6)
                tt(cx, pl(P9, 1), pl(P9, 5), add)               # a1t1+a2t2
                tt(cy, pl(P9, 8), pl(P9, 0), sub)               # a3t2-a1t0
                stt(cz, pl(P9, 7), -1.0, pl(P9, 3), mul, sub)   # -a3t1-a2t0
                # scalar chain (needs ACT outputs, ready by now)
                z = T("z")
                tt(z, es2, S, mul)                  # z = 4 sin^2 th
                fp = T("fp")
                cust(OP_POLY4, fp, z, c1col, s0=FP_C4, s1=FP_C3, imm2=FP_C2)
                qt = T("qt")
                cust(OP_QTH, qt, fp, z, s0=0.25)    # th^2
                g = T("g")
                stt(g, fp, 0.5, es, mul, mul)       # g = f e^{-s}
                # rotation outputs 3..5 = sqrt2 * g * a_k
                stt(ov[:, 3:6, :], bc3(g), SQ2, pl3(A3), mul, mul)
                # psi coefficients
                A = T("e2s"); Bc = T("S"); D = T("lnd2")
                cust(OP_ACOEF, A, s,
                     s0=-1.0 / 720.0, s1=1.0 / 12.0, imm2=-0.5)
                cust(OP_BCOEF, Bc, s, qt,
                     s0=-1.0 / 180.0, s1=1.0 / 6.0, imm2=-0.5)
                cust(OP_DCOEF, D, s, qt,
                     s0=-1.0 / 120.0, s1=1.0 / 720.0, imm2=1.0 / 12.0)
                v2 = T("u"); Ap = T("es2"); Bg = T("s"); Dg2 = T("fp")
                tt(v2, D, qt, mul)
                tt(Ap, A, v2, sub)
                tt(Bg, Bc, g, mul)
                cust(OP_DG2, Dg2, D, g)
                P3 = T("z")
                tt(P3, Dg2, dt, mul)
                # pw = P3*(-a3,+a2,-a1) into free P9 planes 1,3,5
                stt(pl(P9, 1), P3, -1.0, pl(A3, 2), mul, mul)
                tt(pl(P9, 3), P3, pl(A3, 1), mul)
                stt(pl(P9, 5), P3, -1.0, pl(A3, 0), mul, mul)
                # w1 = Ap*t ; w2 = Bg*ctil (into A3, fully consumed by now)
                W1 = T("W1", 3)
                tt(pl3(W1), bc3(Ap), tv, mul)
                # planes {2,4,6} and {1,3,5} of P9 as [p, 3, nf] (step 2*nf)
                cview = P9[:, 2 * nf:8 * nf].rearrange(
                    "p (c t j) -> p c t j", c=3, t=2)[:, :, 0, :]
                pwview = P9[:, 1 * nf:7 * nf].rearrange(
                    "p (c t j) -> p c t j", c=3, t=2)[:, :, 0, :]
                tt(pl3(A3), bc3(Bg), cview, mul)
                tt(pl3(W1), pl3(W1), pl3(A3), add)
                tt(ov[:, 0:3, :], pl3(W1), pwview, add)

                nc.sync.dma_start(out=yv[:, off * 7:(off + nf) * 7], in_=OUT)
                off += nf
    if not nc.is_finalized():
        nc.finalize()
    return nc


def _run(affine, trace=False):
    x = np.ascontiguousarray(affine.reshape(B, 16).astype(np.float32, copy=False))
    pad = NCORES * NC_ELEMS - B
    padblk = np.zeros((pad, 16), np.float32)
    padblk[:, [0, 5, 10, 15]] = 1.0  # identity affines -> log = 0
    data = np.concatenate([x, padblk], 0).reshape(NCORES, NC_ELEMS, 16)
    nc = _build()
    res = run_bass_kernel_spmd(
        nc,
        [{"x": data[i]} for i in range(NCORES)],
        core_ids=list(range(NCORES)),
        trace=trace,
    )
    y = np.concatenate([r["y"] for r in res.results], 0)[:B]
    return y, res


def kernel(affine):
    y, _ = _run(np.asarray(affine), trace=False)
    return y


# revision 10
# speedup vs baseline: 1.9573x; 1.2804x over previous
"""nn_AffineLog: batched 4x4 affine matrix-log projected onto the 7-dim CSO basis.

Closed-form algorithm (replaces the reference's 24-term Mercator series):
inputs are exactly [[e^s R, t],[0,1]] with R a rotation, so
  L3x3 = s I + f (R - R^T),  f = asin(x)/(2x), x = sin th  (poly in x^2)
  translation u' = psi(C) t, psi(x) = x/(e^x-1), reduced via
  Omega^3 = -th^2 Omega to u' = (A - D q) t + B (w x t) + D (w.t) w.

Data-parallel over 8 NeuronCores. The host packs the 10 live channels of
each affine into channel-planar per-partition DRAM, so every DVE access is
contiguous; elementwise pipeline on DVE/ACT with custom fused DVE ops.
"""

import os

os.environ.setdefault("BY_DEFAULT_DISABLE_SUBTILE_DEPS", "1")

import functools

import numpy as np

import concourse.bass as bass
import concourse.bacc as bacc
import concourse.hw_specs as hw_specs
import concourse.mybir as mybir
from concourse.tile import TileContext
from concourse.bass_utils import run_bass_kernel_spmd
from concourse import dve_ops as dops
from concourse.dve_spec import (
    Spec, Src0, Src1, C0, C1, C2, C3, One, sq, _spill_c3_to_src1, lower,
    _has_src1,
)
from concourse.dve_uop import DveOpSpec

AF = mybir.ActivationFunctionType
OP = mybir.AluOpType
F32 = mybir.dt.float32

NCORES = 8
B = 2_000_000
P = 128
JPP = 1954                  # free-dim elements per partition per core
NC_ELEMS = P * JPP          # 250112 per core (total 2000896, pad 896)
TILES = (226, 768, 768, 192)

# packed channel order (host): [m01, m10, m02, m20, m12, m21, m00] + [t0, t1, t2]
CH_A = [1, 4, 2, 8, 6, 9, 0]   # 7 "matrix" planes -> tensor xa
CH_B = [3, 7, 11]              # 3 translation planes -> tensor xb

SQ2 = float(np.sqrt(2.0))
SQ3 = float(np.sqrt(3.0))
# f'(z) = 2*asin(x)/(2x) with z = 4x^2:  f' = 1 + c1 z + c2 z^2 + c3 z^3 + c4 z^4
FP_C1 = 1.0 / 24.0
FP_C2 = 2.0 * 0.5 * (3.0 / 40.0) / 16.0
FP_C3 = 2.0 * 0.5 * (5.0 / 112.0) / 64.0
FP_C4 = 2.0 * 0.5 * (35.0 / 1152.0) / 256.0

# Restrict ACT table choice to the one set holding ln+exp+copy, so bacc
# never alternates table loads between tiles. Other set names stay (ids are
# positional) but advertise no functions.
_orig_gat = hw_specs.get_activation_tables


@functools.cache
def _gat_ln_exp_only(module_arch):
    t = _orig_gat(module_arch)
    keep = "natural_log_exp_and_others"
    return {k: (v if k == keep else set()) for k, v in t.items()}


hw_specs.get_activation_tables = _gat_ln_exp_only
bacc.get_activation_tables = _gat_ln_exp_only


# --- custom fused DVE ops (registered into concourse.dve_ops at import) ----
def _register(name, body):
    if name in dops._SUB_OPCODE_FOR_NAME:
        return next(o for o in dops.OPS if o.name == name)
    dops._SUB_OPCODE_FOR_NAME[name] = dops._CUSTOM_DVE_ROW_BASE + len(dops.OPS)
    assert dops._SUB_OPCODE_FOR_NAME[name] < 0x20
    spec = Spec(body=body)
    lowered = DveOpSpec(
        name=name,
        opcode=dops._SUB_OPCODE_FOR_NAME[name],
        uops=lower(spec, ver="v3"),
        rd1_en=_has_src1(spec),
    )
    op = dops.DveOp(name=name, spec=spec, subdim=False,
                    uops_sha={"v3": lowered.sha("v3")})
    dops.OPS.append(op)
    dops.CUSTOM_DVE_SPECS[name] = spec
    return op


OP_SQSUM = _register("ANT_AFL_SQSUM", sq(Src0) + sq(Src1))
OP_ADDSQ = _register("ANT_AFL_ADDSQ", Src0 + sq(Src1))
OP_POLY4 = _register(
    "ANT_AFL_POLY4",
    _spill_c3_to_src1(((((Src0 * C0 + C1) * Src0 + C2) * Src0 + C3) * Src0) + One),
)
_m2 = (Src0 * C0) * Src0
OP_ACOEF = _register("ANT_AFL_ACOEF", (((_m2 + C1) * Src0 + C2) * Src0) + One)
_s2 = Src0 * Src0
OP_BCOEF = _register(
    "ANT_AFL_BCOEF", (Src0 * C1 + C2) + ((_s2 * Src0 - Src0 * Src1) * C0))
OP_DCOEF = _register(
    "ANT_AFL_DCOEF", ((Src0 * Src0) * C0 + Src1 * C1) + C2)
OP_QTH = _register("ANT_AFL_QTH", (sq(Src0) * Src1) * C0)
OP_DG2 = _register("ANT_AFL_DG2", Src0 * sq(Src1))


def _build(jpp=JPP, tiles=TILES):
    nc = bacc.Bacc("TRN2", target_bir_lowering=False, debug=False)
    xa = nc.dram_tensor("xa", (P, 7 * jpp), F32, kind="ExternalInput")
    xb = nc.dram_tensor("xb", (P, 3 * jpp), F32, kind="ExternalInput")
    ya = nc.dram_tensor("ya", (P, 3 * jpp), F32, kind="ExternalOutput")
    yb = nc.dram_tensor("yb", (P, 4 * jpp), F32, kind="ExternalOutput")
    xav = xa[:, :].rearrange("p (c j) -> p c j", j=jpp)
    xbv = xb[:, :].rearrange("p (c j) -> p c j", j=jpp)
    yav = ya[:, :].rearrange("p (c j) -> p c j", j=jpp)
    ybv = yb[:, :].rearrange("p (c j) -> p c j", j=jpp)

    mul, add, sub = OP.mult, OP.add, OP.subtract

    with TileContext(nc) as tc:
        with (
            tc.tile_pool(name="cst", bufs=1) as cstp,
            tc.tile_pool(name="io", bufs=2) as iop,
            tc.tile_pool(name="tp", bufs=1) as tp,
        ):
            c1col = cstp.tile([P, 1], F32, name="c1col")
            nc.vector.memset(c1col, FP_C1)

            off = 0
            for nf in tiles:
                INA = iop.tile([P, nf * 7], F32, tag="ina", name="tina")
                INB = iop.tile([P, nf * 3], F32, tag="inb", name="tinb")
                OUTA = iop.tile([P, nf * 3], F32, tag="outa", name="touta")
                OUTB = iop.tile([P, nf * 4], F32, tag="outb", name="toutb")
                nc.sync.dma_start(
                    out=INA.rearrange("p (c j) -> p c j", c=7),
                    in_=xav[:, :, off:off + nf])
                nc.sync.dma_start(
                    out=INB.rearrange("p (c j) -> p c j", c=3),
                    in_=xbv[:, :, off:off + nf])

                def T(nm, k=1):
                    return tp.tile([P, nf * k], F32, tag=nm, name=nm)

                def pl(t, i, k=1):
                    return t[:, i * nf:(i + k) * nf]

                def pl3(t, i=0):
                    return t[:, i * nf:(i + 3) * nf].rearrange(
                        "p (c j) -> p c j", c=3)

                def bc3(a):
                    return a.rearrange("p (o j) -> p o j", o=1).to_broadcast(
                        [P, 3, nf])

                def tt(o, a, b, op):
                    nc.vector.tensor_tensor(out=o, in0=a, in1=b, op=op)

                def stt(o, a, s, b, op0, op1):
                    nc.vector.scalar_tensor_tensor(
                        out=o, in0=a, scalar=s, in1=b, op0=op0, op1=op1)

                def cust(op_, o, a, b=None, s0=0.0, s1=0.0, imm2=0.0):
                    nc.vector._custom_dve(
                        op_, out=o, in0=a, in1=b, s0=s0, s1=s1, imm2=imm2)

                tv = pl3(INB)  # [p, 3, nf] translation planes

                u = T("u"); v = T("v")
                # e^{2s} = m00^2 + m10^2 + m20^2  (planes 6, 1, 3 of INA)
                cust(OP_SQSUM, u, pl(INA, 6), pl(INA, 1))
                e2s = T("e2s")
                cust(OP_ADDSQ, e2s, u, pl(INA, 3))
                lnd2 = T("lnd2"); es = T("es"); es2 = T("es2"); s = T("s")
                nc.scalar.activation(out=lnd2, in_=e2s, func=AF.Ln)
                nc.scalar.activation(out=es, in_=lnd2, func=AF.Exp, scale=-0.5)
                nc.scalar.activation(out=es2, in_=lnd2, func=AF.Exp, scale=-1.0)
                nc.scalar.mul(s, lnd2, 0.5)
                nc.scalar.mul(pl(OUTB, 3), lnd2, SQ3 / 2.0)   # out6

                A3 = T("A3", 3)
                tt(pl(A3, 0), pl(INA, 0), pl(INA, 1), sub)   # a1 = m01 - m10
                tt(pl(A3, 1), pl(INA, 2), pl(INA, 3), sub)   # a2 = m02 - m20
                tt(pl(A3, 2), pl(INA, 4), pl(INA, 5), sub)   # a3 = m12 - m21
                cust(OP_SQSUM, v, pl(A3, 0), pl(A3, 1))
                S = T("S")
                cust(OP_ADDSQ, S, v, pl(A3, 2))
                # all 9 products P[i,j] = a_i * t_j at plane 3i+j
                P9 = T("P9", 9)
                for i in range(3):
                    tt(pl3(P9, 3 * i), bc3(pl(A3, i)), tv, mul)
                # dtil first (consumes planes 4,6,2), then ctil into 2,4,6
                dA = T("dA"); dt = T("dt")
                tt(dA, pl(P9, 4), pl(P9, 6), sub)               # a2t1-a3t0
                tt(dt, dA, pl(P9, 2), sub)                      # - a1t2
                tt(pl(P9, 2), pl(P9, 1), pl(P9, 5), add)        # cx
                tt(pl(P9, 4), pl(P9, 8), pl(P9, 0), sub)        # cy
                stt(pl(P9, 6), pl(P9, 7), -1.0, pl(P9, 3), mul, sub)  # cz
                # scalar chain (ACT outputs ready by now)
                z = T("z")
                tt(z, es2, S, mul)                  # z = 4 sin^2 th
                fp = T("fp")
                cust(OP_POLY4, fp, z, c1col, s0=FP_C4, s1=FP_C3, imm2=FP_C2)
                qt = T("qt")
                cust(OP_QTH, qt, fp, z, s0=0.25)    # th^2
                g = T("g")
                stt(g, fp, 0.5, es, mul, mul)       # g = f e^{-s}
                # rotation outputs = sqrt2 * g * a_k -> OUTB planes 0..2
                stt(pl3(OUTB), bc3(g), SQ2, pl3(A3), mul, mul)
                nc.sync.dma_start(
                    out=ybv[:, :, off:off + nf],
                    in_=OUTB.rearrange("p (c j) -> p c j", c=4))
                # psi coefficients (slots reuse dead temps)
                A = T("e2s"); Bc = T("S"); D = T("lnd2")
                cust(OP_ACOEF, A, s,
                     s0=-1.0 / 720.0, s1=1.0 / 12.0, imm2=-0.5)
                cust(OP_BCOEF, Bc, s, qt,
                     s0=-1.0 / 180.0, s1=1.0 / 6.0, imm2=-0.5)
                cust(OP_DCOEF, D, s, qt,
                     s0=-1.0 / 120.0, s1=1.0 / 720.0, imm2=1.0 / 12.0)
                v2 = T("u"); Ap = T("es2"); Bg = T("s"); Dg2 = T("fp")
                tt(v2, D, qt, mul)
                tt(Ap, A, v2, sub)
                tt(Bg, Bc, g, mul)
                cust(OP_DG2, Dg2, D, g)
                P3 = T("z")
                tt(P3, Dg2, dt, mul)
                # pw = P3*(-a3,+a2,-a1) into free P9 planes 1,3,5
                stt(pl(P9, 1), P3, -1.0, pl(A3, 2), mul, mul)
                tt(pl(P9, 3), P3, pl(A3, 1), mul)
                stt(pl(P9, 5), P3, -1.0, pl(A3, 0), mul, mul)
                # w1 = Ap*t ; w2 = Bg*ctil (into A3, fully consumed)
                W1 = T("W1", 3)
                tt(pl3(W1), bc3(Ap), tv, mul)
                cview = P9[:, 2 * nf:8 * nf].rearrange(
                    "p (c t j) -> p c t j", c=3, t=2)[:, :, 0, :]
                pwview = P9[:, 1 * nf:7 * nf].rearrange(
                    "p (c t j) -> p c t j", c=3, t=2)[:, :, 0, :]
                tt(pl3(A3), bc3(Bg), cview, mul)
                tt(pl3(W1), pl3(W1), pl3(A3), add)
                tt(pl3(OUTA), pl3(W1), pwview, add)
                nc.sync.dma_start(
                    out=yav[:, :, off:off + nf],
                    in_=OUTA.rearrange("p (c j) -> p c j", c=3))
                off += nf
    if not nc.is_finalized():
        nc.finalize()
    return nc


def _pack(affine):
    """(B,4,4) f32 -> per-core channel-planar arrays xa (P,7*jpp), xb (P,3*jpp)."""
    x = np.ascontiguousarray(affine.reshape(B, 16).astype(np.float32, copy=False))
    pad = NCORES * NC_ELEMS - B
    padblk = np.zeros((pad, 16), np.float32)
    padblk[:, [0, 5, 10, 15]] = 1.0  # identity affines -> log = 0
    data = np.concatenate([x, padblk], 0).reshape(NCORES, P, JPP, 16)
    da = np.ascontiguousarray(data[:, :, :, CH_A].transpose(0, 1, 3, 2))
    db = np.ascontiguousarray(data[:, :, :, CH_B].transpose(0, 1, 3, 2))
    return (da.reshape(NCORES, P, 7 * JPP), db.reshape(NCORES, P, 3 * JPP))


def _run(affine, trace=False):
    da, db = _pack(np.asarray(affine))
    nc = _build()
    res = run_bass_kernel_spmd(
        nc,
        [{"xa": da[i], "xb": db[i]} for i in range(NCORES)],
        core_ids=list(range(NCORES)),
        trace=trace,
    )
    out = np.empty((NCORES, P, JPP, 7), np.float32)
    for i, r in enumerate(res.results):
        out[i, :, :, 0:3] = r["ya"].reshape(P, 3, JPP).transpose(0, 2, 1)
        out[i, :, :, 3:7] = r["yb"].reshape(P, 4, JPP).transpose(0, 2, 1)
    return out.reshape(NCORES * NC_ELEMS, 7)[:B], res


def kernel(affine):
    y, _ = _run(np.asarray(affine), trace=False)
    return y
